# revision 6
# baseline (speedup 1.0000x reference)
"""Trainium2 kernel for nn_MAg_90709709292194 (gnn_message_passing).

Computation: out = inputs @ ker_wt + bias, where ker_wt (8192x8192) holds the
`kernel` values scattered into the nonzero pattern of tile(adjacency, (4, 4))
in row-major nonzero order. ker_wt is ~0.8% dense, so instead of streaming a
dense 16 MiB weight slice per core (the previous baseline), this kernel works
edge-wise:

  out[b, co*N + j] = sum_{i in N(j)} sum_ci K[e(i,j), ci, co] * X[b, ci*N + i]

Device strategy (8 cores, no collectives):
  - Output nodes j are sharded: core k owns j in [256k, 256(k+1)).
  - Host packs the core's ~4.4k edges (CSC order) into K-tiles of 128 "slots"
    (<= 8 output nodes per tile, a node's edges never split across tiles).
    Each tile's weights form a block-diagonal [128 slots x 32 (j,co)-cols]
    fp16 matrix; cross-terms are zero. Slot 127 of every tile reads a
    constant-ones row so the bias folds in as ordinary weights (ci=0 plane).
  - On device, X is cast+regrouped to a node-major [2048+1, 128=(ci,b)] fp16
    table in DRAM (gpsimd cast DMA + one XBAR transpose), then a single
    gpsimd dma_gather pulls each slot's node row into the matching SBUF
    partition: xg[p, t, :] = XT[idx[t*128+p], :].
  - Per tile t: 4 matmuls (one per ci) accumulate into PSUM band (t%4),
    column block t//4: acc[32*(t%4)+(jl,co), 32*(t//4)+b].
  - DVE evacuates PSUM -> SBUF -> DRAM; the host inverse-permutes
    (tile, jl, co) rows back to output columns.

HBM traffic per core ~ 3.5 MiB (vs 17 MiB dense): X 1 MiB + staging 1 MiB +
gather 1.3 MiB + weights ~1.3 MiB, most of it overlapped.
"""

import numpy as np

N = 2048        # nodes
IN_CHAN = 4
CHANNELS = 4
B = 32          # batch
D = N * IN_CHAN     # 8192 contraction dim
DV = N * CHANNELS   # 8192 output dim
NCORES = 8
JSH = N // NCORES   # 256 output nodes per core
CAP = 127           # data slots per tile (slot 127 = ones/bias slot)
MAXJ = 8            # max output nodes per tile (M = 8*4 = 32 cols)
TPG = 8             # tiles per weight DMA group

_PROGRAM_CACHE = {}


def build_program(nt, debug=False):
    key = (nt, bool(debug))
    if key in _PROGRAM_CACHE:
        return _PROGRAM_CACHE[key]

    import concourse.bass as bass
    import concourse.bacc as bacc
    import concourse.mybir as mybir
    import concourse.tile as tile
    from concourse.library_config import mlp

    f32 = mybir.dt.float32
    f16 = mybir.dt.float16
    i16 = mybir.dt.int16

    ntc = nt // 4        # psum column blocks
    ng = nt // TPG       # weight groups
    nk = nt * 128        # gather slots

    nc = bacc.Bacc(
        "TRN2", target_bir_lowering=False, debug=debug, num_devices=NCORES
    )
    x = nc.dram_tensor("x", [B, IN_CHAN, N], f32, kind="ExternalInput")
    wt = nc.dram_tensor("wt", [ng, 128, TPG * 4 * 32], f16, kind="ExternalInput")
    idxw = nc.dram_tensor("idxw", [128, nk // 16], i16, kind="ExternalInput")
    outp = nc.dram_tensor("outp", [128, ntc * 32], f32, kind="ExternalOutput")
    xh = nc.dram_tensor("xh_scratch", [128, N], f16)          # (ci,b)-major X
    xtd = nc.dram_tensor("xtd_scratch", [N + 16, 128], f16)   # node-major X

    with tile.TileContext(nc) as tc:
        with (
            tc.tile_pool(name="const", bufs=1) as const,
            tc.tile_pool(name="wpool", bufs=8) as wpool,
            tc.tile_pool(name="psum", bufs=1, space=bass.MemorySpace.PSUM) as psum,
        ):
            nc.gpsimd.load_library(mlp)

            # --- X staging ---------------------------------------------------
            # hop 1: cast+regroup  x[b, ci, i] -> xh[(ci,b), i]  (SWDGE cast)
            nc.gpsimd.dma_start(
                out=xh[:].rearrange("(c b) i -> c b i", c=IN_CHAN),
                in_=x[:].transpose([1, 0, 2]),
            )
            # hop 2: XBAR transpose  xh -> xtp[p, q, (ci,b)] = X(node 128q+p)
            xtp = const.tile([128, N // 128, 128], f16)
            nc.sync.dma_start_transpose(out=xtp[:], in_=xh[:])
            # hop 3: node-major table  xtd[i, :] = xtp[i % 128, i // 128, :]
            nc.sync.dma_start(
                out=xtd[0:N].rearrange("(q p) e -> p q e", p=128), in_=xtp[:]
            )
            # ones row for the bias slot
            ones = const.tile([1, 128], f16)
            nc.vector.memset(ones[:], 1.0)
            nc.sync.dma_start(out=xtd[N : N + 1, :], in_=ones[:])

            # --- gather indices & weights ------------------------------------
            idxsb = const.tile([128, nk // 16], i16)
            nc.sync.dma_start(out=idxsb[:], in_=idxw[:])
            wsb = []
            for g in range(ng):
                w = wpool.tile([128, TPG * 4 * 32], f16, tag="wg")
                nc.scalar.dma_start(out=w[:], in_=wt[g])
                wsb.append(w)

            # --- the gather --------------------------------------------------
            # xg[p, t, :] = xtd[idx[t*128 + p], :]
            # Chunked: a single SWDGE gather is capped by the per-DMA-engine
            # descriptor ring (~128 descs/engine = 2048 idxs); 8-tile chunks
            # (1024 idxs, 65 descs/engine) are verified safe.
            xg = const.tile([128, nt, 128], f16)
            CH = 8
            for c in range(nt // CH):
                nc.gpsimd.dma_gather(
                    xg[:, c * CH : (c + 1) * CH, :],
                    xtd[:],
                    idxsb[:, c * (CH * 8) : (c + 1) * (CH * 8)],
                    CH * 128,
                    CH * 128,
                    128,
                )

            # --- block-diagonal matmuls --------------------------------------
            acc = psum.tile([128, ntc * 32], f32)
            for t in range(nt):
                g, tl = divmod(t, TPG)
                band = t % 4
                co0 = (t // 4) * 32
                for ci in range(4):
                    nc.tensor.matmul(
                        acc[32 * band : 32 * (band + 1), co0 : co0 + 32],
                        wsb[g][:, (tl * 4 + ci) * 32 : (tl * 4 + ci + 1) * 32],
                        xg[:, t, ci * 32 : (ci + 1) * 32],
                        start=(ci == 0),
                        stop=(ci == 3),
                        tile_position=(0, 32 * band),
                        skip_group_check=True,
                    )

            # --- evacuate ----------------------------------------------------
            osb = const.tile([128, ntc * 32], f32)
            nc.vector.tensor_copy(osb[:], acc[:])
            nc.sync.dma_start(out=outp[:], in_=osb[:])

    nc.compile()
    _PROGRAM_CACHE[key] = nc
    return nc


def pack_inputs(inputs, adjacency, kernel, bias):
    """Host-side build()-time packing: edge extraction, kernel-value lookup,
    tile packing (block-diagonal weights), gather indices."""
    X = np.ascontiguousarray(
        np.asarray(inputs, dtype=np.float32).reshape(B, IN_CHAN, N)
    )
    A = np.asarray(adjacency) != 0
    kern = np.asarray(kernel, dtype=np.float32)
    bias = np.asarray(bias, dtype=np.float32)

    # edge enumeration in row-major order (matches reference's cumsum order)
    rows, cols = np.nonzero(A)
    nnz = rows.shape[0]
    rnnz = np.bincount(rows, minlength=N).astype(np.int64)
    prefix = np.concatenate([[0], np.cumsum(rnnz)[:-1]])
    krank = np.arange(nnz, dtype=np.int64) - prefix[rows]
    # val16[ci, e, co] = kernel value of edge e for channel pair (ci, co)
    ci_off = (4 * nnz * np.arange(4))[:, None, None]
    base = (4 * prefix[rows] + krank)[None, :, None]
    co_off = np.arange(4)[None, None, :] * rnnz[rows][None, :, None]
    val16 = kern[ci_off + base + co_off].astype(np.float16)  # [4, nnz, 4]

    # CSC: edges sorted by (j, i)
    perm = np.lexsort((rows, cols))
    csc_src = rows[perm]
    csc_dst = cols[perm]
    cdeg = np.bincount(cols, minlength=N)
    cptr = np.concatenate([[0], np.cumsum(cdeg)])

    # --- tile packing per core ---
    per_core = []
    for k in range(NCORES):
        tiles = []  # list of list[(j, e_lo, e_hi)]
        cur, cur_slots = [], 0
        for j in range(JSH * k, JSH * (k + 1)):
            d = int(cdeg[j])
            assert 0 < d <= CAP
            if cur and (cur_slots + d > CAP or len(cur) == MAXJ):
                tiles.append(cur)
                cur, cur_slots = [], 0
            cur.append((j, int(cptr[j]), int(cptr[j + 1])))
            cur_slots += d
        if cur:
            tiles.append(cur)
        per_core.append(tiles)

    nt = max(len(t) for t in per_core)
    nt = -(-nt // TPG) * TPG  # round up to tiles-per-group multiple
    ntc = nt // 4
    ng = nt // TPG

    in_maps, jmaps = [], []
    for k in range(NCORES):
        tiles = per_core[k]
        wtk = np.zeros((nt, 4, 128, 32), np.float16)
        idxk = np.zeros(nt * 128, np.int16)
        jmap = np.full((nt, MAXJ), -1, np.int64)
        for t, tl in enumerate(tiles):
            p = 0
            for jl, (j, elo, ehi) in enumerate(tl):
                d = ehi - elo
                e = perm[elo:ehi]
                idxk[t * 128 + p : t * 128 + p + d] = csc_src[elo:ehi]
                # [4ci, d, 4co] -> cols jl*4+co
                wtk[t, :, p : p + d, jl * 4 : jl * 4 + 4] = val16[:, e, :]
                # bias via the ones slot (ci=0 plane only)
                wtk[t, 0, CAP, jl * 4 : jl * 4 + 4] = bias[
                    np.arange(4) * N + j
                ].astype(np.float16)
                jmap[t, jl] = j
                p += d
            idxk[t * 128 + CAP] = N  # ones row
        # weight groups laid [ng, 128, TPG*4*32] for contiguous group DMAs
        wg = (
            wtk.reshape(ng, TPG, 4, 128, 32)
            .transpose(0, 3, 1, 2, 4)
            .reshape(ng, 128, TPG * 4 * 32)
        )
        idxw = np.tile(idxk.reshape(-1, 16).T, (8, 1))  # [128, nk//16]
        in_maps.append(
            {
                "x": X,
                "wt": np.ascontiguousarray(wg),
                "idxw": np.ascontiguousarray(idxw),
            }
        )
        jmaps.append(jmap)
    return nt, in_maps, jmaps


def unpack_output(nt, jmaps, results):
    """outp[32*(t%4) + 4*jl + co, 32*(t//4) + b] -> out[b, co*N + j]."""
    out = np.zeros((B, DV), np.float32)
    for k in range(NCORES):
        outp = results[k]["outp"]  # [128, (nt//4)*32]
        jmap = jmaps[k]
        t_arr, jl_arr = np.nonzero(jmap >= 0)
        j_arr = jmap[t_arr, jl_arr]
        for co in range(4):
            part = 32 * (t_arr % 4) + 4 * jl_arr + co
            colb = (32 * (t_arr // 4))[:, None] + np.arange(B)[None, :]
            out[:, co * N + j_arr] = outp[part[:, None], colb].T
    return out


def run(nt, in_maps, trace=False, **kwargs):
    from concourse.bass_utils import run_bass_kernel_spmd

    nc = build_program(nt, debug=False)
    res = run_bass_kernel_spmd(
        nc, in_maps, core_ids=list(range(NCORES)), trace=trace, **kwargs
    )
    return res


def run_full(packed, trace=False, **kwargs):
    nt, in_maps, jmaps = packed
    res = run(nt, in_maps, trace=trace, **kwargs)
    return unpack_output(nt, jmaps, res.results), res


def kernel(inputs, adjacency, kernel, bias):
    out, _ = run_full(pack_inputs(inputs, adjacency, kernel, bias))
    return out


# revision 7
# speedup vs baseline: 1.0470x; 1.0470x over previous
"""Trainium2 kernel for nn_MAg_90709709292194 (gnn_message_passing).

Computation: out = inputs @ ker_wt + bias, where ker_wt (8192x8192) holds the
`kernel` values scattered into the nonzero pattern of tile(adjacency, (4, 4))
in row-major nonzero order. ker_wt is ~0.8% dense, so instead of streaming a
dense 16 MiB weight slice per core (the previous baseline), this kernel works
edge-wise:

  out[b, co*N + j] = sum_{i in N(j)} sum_ci K[e(i,j), ci, co] * X[b, ci*N + i]

Device strategy (8 cores, no collectives):
  - Output nodes j are sharded: core k owns j in [256k, 256(k+1)).
  - Host packs the core's ~4.4k edges (CSC order) into K-tiles of 128 "slots"
    (<= 8 output nodes per tile, a node's edges never split across tiles).
    Each tile's weights form a block-diagonal [128 slots x 32 (j,co)-cols]
    fp16 matrix; cross-terms are zero. Slot 127 of every tile reads a
    constant-ones row so the bias folds in as ordinary weights (ci=0 plane).
  - On device, X is cast+regrouped to a node-major [2048+1, 128=(ci,b)] fp16
    table in DRAM (gpsimd cast DMA + one XBAR transpose), then a single
    gpsimd dma_gather pulls each slot's node row into the matching SBUF
    partition: xg[p, t, :] = XT[idx[t*128+p], :].
  - Per tile t: 4 matmuls (one per ci) accumulate into PSUM band (t%4),
    column block t//4: acc[32*(t%4)+(jl,co), 32*(t//4)+b].
  - DVE evacuates PSUM -> SBUF -> DRAM; the host inverse-permutes
    (tile, jl, co) rows back to output columns.

HBM traffic per core ~ 3.5 MiB (vs 17 MiB dense): X 1 MiB + staging 1 MiB +
gather 1.3 MiB + weights ~1.3 MiB, most of it overlapped.
"""

import numpy as np

N = 2048        # nodes
IN_CHAN = 4
CHANNELS = 4
B = 32          # batch
D = N * IN_CHAN     # 8192 contraction dim
DV = N * CHANNELS   # 8192 output dim
NCORES = 8
JSH = N // NCORES   # 256 output nodes per core
CAP = 127           # data slots per tile (slot 127 = ones/bias slot)
MAXJ = 8            # max output nodes per tile (M = 8*4 = 32 cols)
TPG = 8             # tiles per weight DMA group

_PROGRAM_CACHE = {}


def build_program(nt, debug=False):
    key = (nt, bool(debug))
    if key in _PROGRAM_CACHE:
        return _PROGRAM_CACHE[key]

    import concourse.bass as bass
    import concourse.bacc as bacc
    import concourse.mybir as mybir
    import concourse.tile as tile
    from concourse.library_config import mlp

    f32 = mybir.dt.float32
    f16 = mybir.dt.float16
    i16 = mybir.dt.int16

    ntc = nt // 4        # psum column blocks
    ng = nt // TPG       # weight groups
    nk = nt * 128        # gather slots

    nc = bacc.Bacc(
        "TRN2", target_bir_lowering=False, debug=debug, num_devices=NCORES
    )
    x = nc.dram_tensor("x", [B, IN_CHAN, N], f32, kind="ExternalInput")
    wt = nc.dram_tensor("wt", [ng, 128, TPG * 4 * 32], f16, kind="ExternalInput")
    idxw = nc.dram_tensor("idxw", [128, nk // 16], i16, kind="ExternalInput")
    outp = nc.dram_tensor("outp", [128, ntc * 32], f32, kind="ExternalOutput")
    xh = nc.dram_tensor("xh_scratch", [128, N], f16)          # (ci,b)-major X
    xtd = nc.dram_tensor("xtd_scratch", [N + 16, 128], f16)   # node-major X

    with tile.TileContext(nc) as tc:
        with (
            tc.tile_pool(name="const", bufs=1) as const,
            tc.tile_pool(name="wpool", bufs=8) as wpool,
            tc.tile_pool(name="psum", bufs=1, space=bass.MemorySpace.PSUM) as psum,
        ):
            # --- X staging ---------------------------------------------------
            # hop 1: cast+regroup  x[b, ci, i] -> xh[(ci,b), i]  (SWDGE cast).
            # Issued before load_library so the ~13us ucode reload overlaps
            # the cast transfers and the sync-queue staging hops.
            nc.gpsimd.dma_start(
                out=xh[:].rearrange("(c b) i -> c b i", c=IN_CHAN),
                in_=x[:].transpose([1, 0, 2]),
            )
            nc.gpsimd.load_library(mlp)
            # hop 2: XBAR transpose  xh -> xtp[p, q, (ci,b)] = X(node 128q+p)
            xtp = const.tile([128, N // 128, 128], f16)
            nc.sync.dma_start_transpose(out=xtp[:], in_=xh[:])
            # hop 3: node-major table  xtd[i, :] = xtp[i % 128, i // 128, :]
            nc.sync.dma_start(
                out=xtd[0:N].rearrange("(q p) e -> p q e", p=128), in_=xtp[:]
            )
            # ones row for the bias slot
            ones = const.tile([1, 128], f16)
            nc.vector.memset(ones[:], 1.0)
            nc.sync.dma_start(out=xtd[N : N + 1, :], in_=ones[:])

            # --- gather indices & weights ------------------------------------
            idxsb = const.tile([128, nk // 16], i16)
            nc.sync.dma_start(out=idxsb[:], in_=idxw[:])
            wsb = []
            for g in range(ng):
                w = wpool.tile([128, TPG * 4 * 32], f16, tag="wg")
                nc.scalar.dma_start(out=w[:], in_=wt[g])
                wsb.append(w)

            # --- the gather --------------------------------------------------
            # xg[p, t, :] = xtd[idx[t*128 + p], :]
            # Chunked: a single SWDGE gather is capped by the per-DMA-engine
            # descriptor ring (~128 descs/engine = 2048 idxs); 8-tile chunks
            # (1024 idxs, 65 descs/engine) are verified safe.
            xg = const.tile([128, nt, 128], f16)
            CH = 8
            for c in range(nt // CH):
                nc.gpsimd.dma_gather(
                    xg[:, c * CH : (c + 1) * CH, :],
                    xtd[:],
                    idxsb[:, c * (CH * 8) : (c + 1) * (CH * 8)],
                    CH * 128,
                    CH * 128,
                    128,
                )

            # --- block-diagonal matmuls --------------------------------------
            acc = psum.tile([128, ntc * 32], f32)
            for t in range(nt):
                g, tl = divmod(t, TPG)
                band = t % 4
                co0 = (t // 4) * 32
                for ci in range(4):
                    nc.tensor.matmul(
                        acc[32 * band : 32 * (band + 1), co0 : co0 + 32],
                        wsb[g][:, (tl * 4 + ci) * 32 : (tl * 4 + ci + 1) * 32],
                        xg[:, t, ci * 32 : (ci + 1) * 32],
                        start=(ci == 0),
                        stop=(ci == 3),
                        tile_position=(0, 32 * band),
                        skip_group_check=True,
                    )

            # --- evacuate ----------------------------------------------------
            osb = const.tile([128, ntc * 32], f32)
            nc.vector.tensor_copy(osb[:], acc[:])
            nc.sync.dma_start(out=outp[:], in_=osb[:])

    nc.compile()
    _PROGRAM_CACHE[key] = nc
    return nc


def pack_inputs(inputs, adjacency, kernel, bias):
    """Host-side build()-time packing: edge extraction, kernel-value lookup,
    tile packing (block-diagonal weights), gather indices."""
    X = np.ascontiguousarray(
        np.asarray(inputs, dtype=np.float32).reshape(B, IN_CHAN, N)
    )
    A = np.asarray(adjacency) != 0
    kern = np.asarray(kernel, dtype=np.float32)
    bias = np.asarray(bias, dtype=np.float32)

    # edge enumeration in row-major order (matches reference's cumsum order)
    rows, cols = np.nonzero(A)
    nnz = rows.shape[0]
    rnnz = np.bincount(rows, minlength=N).astype(np.int64)
    prefix = np.concatenate([[0], np.cumsum(rnnz)[:-1]])
    krank = np.arange(nnz, dtype=np.int64) - prefix[rows]
    # val16[ci, e, co] = kernel value of edge e for channel pair (ci, co)
    ci_off = (4 * nnz * np.arange(4))[:, None, None]
    base = (4 * prefix[rows] + krank)[None, :, None]
    co_off = np.arange(4)[None, None, :] * rnnz[rows][None, :, None]
    val16 = kern[ci_off + base + co_off].astype(np.float16)  # [4, nnz, 4]

    # CSC: edges sorted by (j, i)
    perm = np.lexsort((rows, cols))
    csc_src = rows[perm]
    csc_dst = cols[perm]
    cdeg = np.bincount(cols, minlength=N)
    cptr = np.concatenate([[0], np.cumsum(cdeg)])

    # --- tile packing per core ---
    per_core = []
    for k in range(NCORES):
        tiles = []  # list of list[(j, e_lo, e_hi)]
        cur, cur_slots = [], 0
        for j in range(JSH * k, JSH * (k + 1)):
            d = int(cdeg[j])
            assert 0 < d <= CAP
            if cur and (cur_slots + d > CAP or len(cur) == MAXJ):
                tiles.append(cur)
                cur, cur_slots = [], 0
            cur.append((j, int(cptr[j]), int(cptr[j + 1])))
            cur_slots += d
        if cur:
            tiles.append(cur)
        per_core.append(tiles)

    nt = max(len(t) for t in per_core)
    nt = -(-nt // TPG) * TPG  # round up to tiles-per-group multiple
    ntc = nt // 4
    ng = nt // TPG

    in_maps, jmaps = [], []
    for k in range(NCORES):
        tiles = per_core[k]
        wtk = np.zeros((nt, 4, 128, 32), np.float16)
        idxk = np.zeros(nt * 128, np.int16)
        jmap = np.full((nt, MAXJ), -1, np.int64)
        for t, tl in enumerate(tiles):
            p = 0
            for jl, (j, elo, ehi) in enumerate(tl):
                d = ehi - elo
                e = perm[elo:ehi]
                idxk[t * 128 + p : t * 128 + p + d] = csc_src[elo:ehi]
                # [4ci, d, 4co] -> cols jl*4+co
                wtk[t, :, p : p + d, jl * 4 : jl * 4 + 4] = val16[:, e, :]
                # bias via the ones slot (ci=0 plane only)
                wtk[t, 0, CAP, jl * 4 : jl * 4 + 4] = bias[
                    np.arange(4) * N + j
                ].astype(np.float16)
                jmap[t, jl] = j
                p += d
            idxk[t * 128 + CAP] = N  # ones row
        # weight groups laid [ng, 128, TPG*4*32] for contiguous group DMAs
        wg = (
            wtk.reshape(ng, TPG, 4, 128, 32)
            .transpose(0, 3, 1, 2, 4)
            .reshape(ng, 128, TPG * 4 * 32)
        )
        idxw = np.tile(idxk.reshape(-1, 16).T, (8, 1))  # [128, nk//16]
        in_maps.append(
            {
                "x": X,
                "wt": np.ascontiguousarray(wg),
                "idxw": np.ascontiguousarray(idxw),
            }
        )
        jmaps.append(jmap)
    return nt, in_maps, jmaps


def unpack_output(nt, jmaps, results):
    """outp[32*(t%4) + 4*jl + co, 32*(t//4) + b] -> out[b, co*N + j]."""
    out = np.zeros((B, DV), np.float32)
    for k in range(NCORES):
        outp = results[k]["outp"]  # [128, (nt//4)*32]
        jmap = jmaps[k]
        t_arr, jl_arr = np.nonzero(jmap >= 0)
        j_arr = jmap[t_arr, jl_arr]
        for co in range(4):
            part = 32 * (t_arr % 4) + 4 * jl_arr + co
            colb = (32 * (t_arr // 4))[:, None] + np.arange(B)[None, :]
            out[:, co * N + j_arr] = outp[part[:, None], colb].T
    return out


def run(nt, in_maps, trace=False, **kwargs):
    from concourse.bass_utils import run_bass_kernel_spmd

    nc = build_program(nt, debug=False)
    res = run_bass_kernel_spmd(
        nc, in_maps, core_ids=list(range(NCORES)), trace=trace, **kwargs
    )
    return res


def run_full(packed, trace=False, **kwargs):
    nt, in_maps, jmaps = packed
    res = run(nt, in_maps, trace=trace, **kwargs)
    return unpack_output(nt, jmaps, res.results), res


def kernel(inputs, adjacency, kernel, bias):
    out, _ = run_full(pack_inputs(inputs, adjacency, kernel, bias))
    return out
